# revision 4
# baseline (speedup 1.0000x reference)
import math
from contextlib import ExitStack

import numpy as np

N, T, D, H = 512, 128, 512, 512
NC = 8
n = N // NC          # 64 samples per core
H4 = 4 * H           # 2048
SCALE = 1.0 / math.sqrt(H)

_cache = {}


def _build_kernel():
    if "nc" in _cache:
        return _cache["nc"]

    import concourse.bass as bass
    import concourse.tile as tile
    from concourse import bacc, mybir

    f32 = mybir.dt.float32
    bf16 = mybir.dt.bfloat16
    ALU = mybir.AluOpType
    ACTF = mybir.ActivationFunctionType

    nc = bacc.Bacc(
        "TRN2",
        target_bir_lowering=False,
        debug=False,
        enable_asserts=False,
        num_devices=NC,
    )

    # ---- DRAM I/O ---------------------------------------------------------
    xTd = nc.dram_tensor("xT", (128, 4 * 8192), bf16, kind="ExternalInput").ap()
    Wxcd = nc.dram_tensor("Wxc", (128, 4 * H4), bf16, kind="ExternalInput").ap()
    Wcd = nc.dram_tensor("Wc", (128, 8 * H4), bf16, kind="ExternalInput").ap()
    ATd = nc.dram_tensor("ATl", (128, 4 * 1024), bf16, kind="ExternalInput").ap()
    Afd = nc.dram_tensor("Af", (128, 8 * 512), bf16, kind="ExternalInput").ap()
    maskd = nc.dram_tensor("maskbd", (n, 1024), bf16, kind="ExternalInput").ap()
    id64d = nc.dram_tensor("id64", (n, n), bf16, kind="ExternalInput").ap()
    injWd = nc.dram_tensor("injW", (n + 1, n), bf16, kind="ExternalInput").ap()
    h0Td = nc.dram_tensor("h0T", (128, 4 * n), bf16, kind="ExternalInput").ap()
    c0d = nc.dram_tensor("c0", (n, H), f32, kind="ExternalInput").ap()
    browd = nc.dram_tensor("brow", (1, H4), bf16, kind="ExternalInput").ap()
    hso = nc.dram_tensor("hso", (T, n, H), bf16, kind="ExternalOutput").ap()

    with tile.TileContext(nc) as tc, ExitStack() as ctx:
        cpool = ctx.enter_context(tc.tile_pool(name="const", bufs=1))
        xtpool = ctx.enter_context(tc.tile_pool(name="xt", bufs=1))
        xwpool = ctx.enter_context(tc.tile_pool(name="xw", bufs=3))
        xbpool = ctx.enter_context(tc.tile_pool(name="xb", bufs=3))
        work = ctx.enter_context(tc.tile_pool(name="work", bufs=2))
        stat = ctx.enter_context(tc.tile_pool(name="stat", bufs=2))
        dpool = ctx.enter_context(tc.tile_pool(name="dram", bufs=1, space="DRAM"))
        p_hb = ctx.enter_context(tc.tile_pool(name="p_hb", bufs=1, space="PSUM"))
        p_of = ctx.enter_context(tc.tile_pool(name="p_of", bufs=1, space="PSUM"))
        p_sm = ctx.enter_context(tc.tile_pool(name="p_sm", bufs=2, space="PSUM"))

        # ---- persistent constants ----------------------------------------
        Wc_sb = cpool.tile([128, 8 * H4], bf16)
        nc.sync.dma_start(Wc_sb[:], Wcd[:])
        Wxc_sb = cpool.tile([128, 4 * H4], bf16)
        nc.sync.dma_start(Wxc_sb[:], Wxcd[:])
        AT_sb = cpool.tile([128, 4 * 1024], bf16)
        nc.sync.dma_start(AT_sb[:], ATd[:])
        Af_sb = cpool.tile([128, 8 * 512], bf16)
        nc.sync.dma_start(Af_sb[:], Afd[:])
        mask_sb = cpool.tile([n, 1024], bf16)
        nc.sync.dma_start(mask_sb[:], maskd[:])
        id_sb = cpool.tile([n, n], bf16)
        nc.sync.dma_start(id_sb[:], id64d[:])
        injW_sb = cpool.tile([n + 1, n], bf16)
        nc.sync.dma_start(injW_sb[:], injWd[:])
        hT = cpool.tile([128, 4 * n], bf16)
        nc.sync.dma_start(hT[:], h0Td[:])
        c_st = cpool.tile([n, H], f32)
        nc.sync.dma_start(c_st[:], c0d[:])

        # ---- phase 1: xwx[r, :] = x_flat[r, :] @ Wx  (+0; b handled later)
        xT_sb = xtpool.tile([128, 4 * 8192], bf16)
        nc.sync.dma_start(xT_sb[:], xTd[:])
        xwx_dram = dpool.tile([64 * T, H4], bf16)

        for m in range(64):
            ps = p_hb.tile([128, H4], f32, tag="hb")
            for k in range(4):
                lhs = xT_sb[:, 8192 * k + 128 * m:8192 * k + 128 * (m + 1)]
                for j in range(4):
                    nc.tensor.matmul(
                        ps[:, 512 * j:512 * (j + 1)],
                        lhs,
                        Wxc_sb[:, H4 * k + 512 * j:H4 * k + 512 * (j + 1)],
                        start=(k == 0),
                        stop=(k == 3),
                    )
            xw_sb = xwpool.tile([128, H4], bf16, tag="xw")
            nc.scalar.activation(xw_sb[:, 0:1024], ps[:, 0:1024], ACTF.Copy)
            nc.vector.tensor_copy(xw_sb[:, 1024:2048], ps[:, 1024:2048])
            nc.sync.dma_start(xwx_dram[128 * m:128 * (m + 1)], xw_sb[:])

        # ---- loop prologue ------------------------------------------------
        xq = {}
        for s in (0, 1, 2):
            xq[s] = xbpool.tile([n + 1, H4], bf16, tag="xq", name=f"xq{s}")
            nc.sync.dma_start(xq[s][0:n], xwx_dram[n * s:n * (s + 1)])
            nc.sync.dma_start(xq[s][n:n + 1], browd[:])

        def inject(hb, xqt):
            for j in range(4):
                nc.tensor.matmul(
                    hb[:, 512 * j:512 * (j + 1)],
                    injW_sb[:],
                    xqt[:, 512 * j:512 * (j + 1)],
                    start=True,
                    stop=False,
                )

        hb = p_hb.tile([n, H4], f32, tag="hb")
        inject(hb, xq[0])

        # ---- recurrence ---------------------------------------------------
        for t in range(T):
            if t + 3 < T:
                xq[t + 3] = xbpool.tile([n + 1, H4], bf16, tag="xq",
                                        name=f"xq{t + 3}")
                nc.sync.dma_start(xq[t + 3][0:n],
                                  xwx_dram[n * (t + 3):n * (t + 4)])
                nc.sync.dma_start(xq[t + 3][n:n + 1], browd[:])

            # out_full[s, (s', p)] = sum_h h[s, h] * A_flat[s', p, h]
            of = p_of.tile([n, 1024], f32, tag="of")
            for k in range(4):
                for g in range(2):
                    nc.tensor.matmul(
                        of[:, 512 * g:512 * (g + 1)],
                        hT[:, n * k:n * (k + 1)],
                        AT_sb[:, 1024 * k + 512 * g:1024 * k + 512 * (g + 1)],
                        start=(k == 0),
                        stop=(k == 3),
                    )

            # h-part of main matmul, chunks 0-1 (fills PE while ACT/DVE work)
            for k in range(2):
                for j in range(4):
                    nc.tensor.matmul(
                        hb[:, 512 * j:512 * (j + 1)],
                        hT[:, n * k:n * (k + 1)],
                        Wc_sb[:, H4 * k + 512 * j:H4 * k + 512 * (j + 1)],
                        start=False,
                        stop=False,
                    )

            # softmax numerator (scale folded into exp; no max-subtraction:
            # |dot*scale| <= ~2.3 for this problem)
            wexp = work.tile([n, 1024], bf16, tag="wexp")
            nc.scalar.activation(wexp[:], of[:], ACTF.Exp, scale=SCALE)

            # block-diagonal mask + row-sum (softmax denominator) in one op
            wexp_m = work.tile([n, 1024], bf16, tag="wexp_m")
            ssum = stat.tile([n, 1], f32, tag="ssum")
            nc.vector.scalar_tensor_tensor(
                wexp_m[:], wexp[:], 1.0, mask_sb[:],
                ALU.mult, ALU.mult, accum_out=ssum[:])

            # transpose wexp_m -> wexpT [(s,p) chunks, s]
            wT = p_sm.tile([128, 512], bf16, tag="sm")
            for cch in range(8):
                nc.tensor.transpose(
                    wT[:, n * cch:n * (cch + 1)],
                    wexp_m[:, 128 * cch:128 * (cch + 1)],
                    id_sb[:])

            # h-part chunk 2
            for j in range(4):
                nc.tensor.matmul(
                    hb[:, 512 * j:512 * (j + 1)],
                    hT[:, n * 2:n * 3],
                    Wc_sb[:, H4 * 2 + 512 * j:H4 * 2 + 512 * (j + 1)],
                    start=False, stop=False)

            wexpT = work.tile([128, 512], bf16, tag="wexpT")
            nc.vector.tensor_copy(wexpT[:], wT[:])
            rinv = stat.tile([n, 1], f32, tag="rinv")
            nc.vector.reciprocal(rinv[:], ssum[:])

            # attn (unnormalized) = wexp_m @ A_flat
            aa = p_sm.tile([n, 512], f32, tag="sm")
            for cch in range(8):
                nc.tensor.matmul(
                    aa[:],
                    wexpT[:, n * cch:n * (cch + 1)],
                    Af_sb[:, 512 * cch:512 * (cch + 1)],
                    start=(cch == 0),
                    stop=(cch == 7),
                )

            # h-part chunk 3
            for j in range(4):
                nc.tensor.matmul(
                    hb[:, 512 * j:512 * (j + 1)],
                    hT[:, n * 3:n * 4],
                    Wc_sb[:, H4 * 3 + 512 * j:H4 * 3 + 512 * (j + 1)],
                    start=False, stop=False)

            # normalize by 1/ssum during PSUM->SBUF copy (per-partition scale)
            attn_n = work.tile([n, H], bf16, tag="attn_n")
            nc.scalar.activation(attn_n[:], aa[:], ACTF.Copy, scale=rinv[:])

            # transpose attn -> attnT
            at = p_sm.tile([128, 4 * n], bf16, tag="sm")
            for k in range(4):
                nc.tensor.transpose(
                    at[:, n * k:n * (k + 1)],
                    attn_n[:, 128 * k:128 * (k + 1)],
                    id_sb[:])
            attnT = work.tile([128, 4 * n], bf16, tag="attnT")
            nc.vector.tensor_copy(attnT[:], at[:])

            # attn-part of main matmul
            for k in range(4):
                for j in range(4):
                    nc.tensor.matmul(
                        hb[:, 512 * j:512 * (j + 1)],
                        attnT[:, n * k:n * (k + 1)],
                        Wc_sb[:, H4 * (4 + k) + 512 * j:H4 * (4 + k) + 512 * (j + 1)],
                        start=False,
                        stop=(k == 3),
                    )

            # gates: column order is [i, f, g, o]
            gif = work.tile([n, 1024], f32, tag="gif")
            nc.scalar.activation(gif[:], hb[:, 0:1024], ACTF.Sigmoid)
            g_t = work.tile([n, H], f32, tag="g_t")
            nc.scalar.activation(g_t[:], hb[:, 1024:1536], ACTF.Tanh)
            o_t = work.tile([n, H], f32, tag="o_t")
            nc.scalar.activation(o_t[:], hb[:, 1536:2048], ACTF.Sigmoid)

            t1 = work.tile([n, H], f32, tag="t1")
            t2 = work.tile([n, H], f32, tag="t2")
            nc.vector.tensor_mul(t1[:], gif[:, 512:1024], c_st[:])
            nc.vector.tensor_mul(t2[:], gif[:, 0:512], g_t[:])
            nc.vector.tensor_add(c_st[:], t1[:], t2[:])
            ct = work.tile([n, H], f32, tag="ct")
            nc.scalar.activation(ct[:], c_st[:], ACTF.Tanh)
            h_bf = work.tile([n, H], bf16, tag="h_bf")
            nc.vector.tensor_mul(h_bf[:], o_t[:], ct[:])

            nc.sync.dma_start(hso[t], h_bf[:])

            # hT for next step
            if t + 1 < T:
                hTp = p_sm.tile([128, 4 * n], bf16, tag="sm")
                for k in range(4):
                    nc.tensor.transpose(
                        hTp[:, n * k:n * (k + 1)],
                        h_bf[:, 128 * k:128 * (k + 1)],
                        id_sb[:])
                nc.vector.tensor_copy(hT[:], hTp[:])

                # start next step's psum with xwx + b
                hb = p_hb.tile([n, H4], f32, tag="hb")
                inject(hb, xq[t + 1])

    nc.compile()
    _cache["nc"] = nc
    return nc


def _prep_host(x, A, Wx, Wh, Wattn, b):
    import ml_dtypes
    bft = ml_dtypes.bfloat16

    # gate column order [i, f, g, o] (reference is [i, f, o, g])
    perm = np.concatenate([
        np.arange(0, 1024),
        np.arange(1536, 2048),
        np.arange(1024, 1536),
    ])
    Wxp = np.asarray(Wx, np.float32)[:, perm]
    Wcat = np.concatenate(
        [np.asarray(Wh, np.float32)[:, perm],
         np.asarray(Wattn, np.float32)[:, perm]], axis=0)     # (1024, 2048)
    bp = np.asarray(b, np.float32)[perm]

    Wc_host = np.ascontiguousarray(
        Wcat.reshape(8, 128, H4).transpose(1, 0, 2).reshape(128, 8 * H4)
    ).astype(bft)
    Wxc_host = np.ascontiguousarray(
        Wxp.reshape(4, 128, H4).transpose(1, 0, 2).reshape(128, 4 * H4)
    ).astype(bft)
    brow = bp.reshape(1, H4).astype(bft)
    id64 = np.eye(n, dtype=np.float32).astype(bft)
    injW = np.concatenate(
        [np.eye(n, dtype=np.float32),
         np.ones((1, n), np.float32)], axis=0).astype(bft)
    maskbd = np.kron(np.eye(n, dtype=np.float32),
                     np.ones((1, 16), np.float32)).astype(bft)

    in_maps = []
    for k in range(NC):
        xc = np.asarray(x[n * k:n * (k + 1)], np.float32)     # (64, T, D)
        Ac = np.asarray(A[n * k:n * (k + 1)], np.float32)     # (64, H, 4, 4)

        x_flat = xc.transpose(1, 0, 2).reshape(T * n, D)      # r = t*64+s
        xT_host = np.ascontiguousarray(
            x_flat.T.reshape(4, 128, T * n).transpose(1, 0, 2)
            .reshape(128, 4 * T * n)).astype(bft)

        A_flat = Ac.reshape(n, H, 16).transpose(0, 2, 1)      # (64, 16, H)
        A_rows = np.ascontiguousarray(A_flat.reshape(n * 16, H))
        AT_host = np.ascontiguousarray(
            A_rows.T.reshape(4, 128, 1024).transpose(1, 0, 2)
            .reshape(128, 4 * 1024)).astype(bft)
        Af_host = np.ascontiguousarray(
            A_rows.reshape(8, 128, H).transpose(1, 0, 2)
            .reshape(128, 8 * H)).astype(bft)

        h0 = Ac.mean(axis=(2, 3)).astype(np.float32)          # (64, 512)
        h0T_host = np.ascontiguousarray(
            h0.T.reshape(4, 128, n).transpose(1, 0, 2)
            .reshape(128, 4 * n)).astype(bft)

        in_maps.append({
            "xT": xT_host,
            "Wxc": Wxc_host,
            "Wc": Wc_host,
            "ATl": AT_host,
            "Af": Af_host,
            "maskbd": maskbd,
            "id64": id64,
            "injW": injW,
            "h0T": h0T_host,
            "c0": h0,
            "brow": brow,
        })
    return in_maps


def kernel(x, A, Wx, Wh, Wattn, b):
    from concourse import bass_utils

    nc = _build_kernel()
    in_maps = _prep_host(x, A, Wx, Wh, Wattn, b)
    res = bass_utils.run_bass_kernel_spmd(nc, in_maps, core_ids=list(range(NC)))

    out = np.empty((N, T, H), dtype=np.float32)
    for k in range(NC):
        hs_k = np.asarray(res.results[k]["hso"]).astype(np.float32)  # (T, n, H)
        out[n * k:n * (k + 1)] = hs_k.transpose(1, 0, 2)
    return out
